# revision 63
# baseline (speedup 1.0000x reference)
"""Trainium2 Bass kernel for nn_LocalCausalSelfAttention (fused QKV + QK-RMSNorm
+ RoPE + causal attention + output projection), data-parallel over the 16
(batch, group) pairs across 8 NeuronCores (2 pairs per core).

Self-contained: kernel(**inputs) takes the FULL inputs and returns the FULL
output.

Per-core dataflow (feature-partition attention — no activation transposes):
  xT  = PE-transpose(x_bf16) via identity matmuls  [E, tok]      (bf16)
  qkT = Wqk-as-lhsT @ xT                           [2E, tok]     (psum f32)
        -> ACT squares -> weighted-ones matmul over partitions -> rms sums
        -> RoPE: pair-swap via PE permutation matmul + 2 mults + add -> bf16
  s_q = rsqrt(mean+eps) row, PE-broadcast, one in-place mult per q tile
  s_k/8 applied for free through exp's per-partition scale operand
        (rsqrt row PE-transposed to per-kv columns)
  v   = xT-as-lhsT @ Wv                            [tok, E]      (bf16,
        stored with a ones-column per head -> softmax denominators for free)
  S^T = kT-as-lhsT @ qT per head (causal-trimmed)  [kv, qt]      (psum f32)
        -> diag-block mask add -> exp (ACT, scale=s_k/8) -> P^T bf16
  o^T = v-as-lhsT @ P^T (accumulate over kv tiles) [65, qt]      row 64 = denom
        -> reciprocal -> PE ones-matmul broadcast -> normalize -> o^T bf16
  y   = o^T-as-lhsT @ Wp                           [tok, E]      (psum f32 -> DRAM)

The two groups are software-pipelined: attention(g0) overlaps the qk
projection of g1, attention(g1) overlaps the output projection of g0.
"""

import sys

sys.path.insert(0, "/opt/trn_rl_repo")

import numpy as np
import ml_dtypes

import concourse.bass as bass
import concourse.tile as tile
from concourse import mybir
from concourse import bass_utils
from concourse.vector_clock import ScopedClock

B, NG, LS, E = 2, 8, 512, 1024
NH, HD = 16, 64
EPS = 1e-6
P = 128
TOK = 2 * LS  # tokens per core (2 (b,g) pairs)
N_CORES = 8
F32 = mybir.dt.float32
BF16 = mybir.dt.bfloat16
NEG_BIG = -1e30

_cache = {}


def _patch_tail_drain():
    """This walrus build supports a single sync-wait per CTRL NO instruction;
    Tile's tail drain aggregates one wait per logical processor. Split the
    waits across single-wait sync nops emitted just before the drain."""
    if getattr(tile.TileContext, "_drain_patched", False):
        return

    def _patched(self, tick_clock, wait_clock):
        nc = self.nc
        probe = nc.sync.nop()
        wait_clock.add_sem_waits(probe.ins, ScopedClock({None: tick_clock.global_clock}))
        si = probe.ins.sync_info
        waits = list(si.on_wait) if si and si.on_wait else []
        if len(waits) > 1:
            si.on_wait = [waits[0]]
            for w in waits[1:]:
                extra = nc.sync.nop()
                if extra.ins.sync_info is None:
                    extra.ins.sync_info = mybir.SyncInfo(on_wait=[w], on_update=[])
                else:
                    extra.ins.sync_info.on_wait = [w]
        nc.sync.drain()
        nc.all_engine_barrier()
        assert self.sems is not None
        popped = nc._tile_sem_poison_stack.pop()
        assert popped is self._sem_poison
        nc.clear_and_free_semaphores(list(self.sems.allocated().values()))
        nc.all_engine_barrier()

    tile.TileContext._drain_and_barrier = _patched
    tile.TileContext._drain_patched = True


def _rope_tables():
    """cos/sin tables matching the reference RoPE variant, transposed to
    [HD, LS] and duplicated for the 2 heads that share a 128-partition tile.
    ss2 carries the rotate() sign pattern: even rows -sin, odd rows +sin."""
    j = np.arange(0, HD, 2, dtype=np.float64)  # 32 freqs
    pos = 10000.0 ** ((-2.0 * j - 1.0) / HD)
    t = np.arange(LS, dtype=np.float64)[:, None]
    token_seq = t * pos[None, :]  # [LS, 32]
    rot = np.concatenate([token_seq, token_seq], axis=-1)  # [LS, HD]
    cos = np.cos(rot).T.astype(np.float32)  # [HD, LS]
    sin = np.sin(rot).T.astype(np.float32)
    sign = np.where(np.arange(HD) % 2 == 0, -1.0, 1.0).astype(np.float32)[:, None]
    ss = sin * sign
    cos2 = np.concatenate([cos, cos], axis=0)  # [128, LS]
    ss2 = np.concatenate([ss, ss], axis=0)
    return np.ascontiguousarray(cos2), np.ascontiguousarray(ss2)


def _split_multi_waits(nc, nop_cls):
    """This walrus build packs sync waits into single-slot instruction structs
    for DMA/CTRL opcodes. Hoist all-but-one wait onto fresh single-wait NoOps
    inserted just before the offending instruction (same engine stream)."""
    seq = 0
    for b in nc.m.functions[0].blocks:
        newl = []
        for inst in b.instructions:
            si = inst.sync_info
            waits = list(si.on_wait) if si and si.on_wait else []
            if len(waits) > 1:
                for w in waits[:-1]:
                    seq += 1
                    nop = nop_cls(name=f"I-waitsplit-{seq}", ins=[], outs=[])
                    nop.engine = inst.engine
                    nop.sync_info = mybir.SyncInfo(on_wait=[w], on_update=[])
                    try:
                        nc.register_instruction(nop, overwrite=True)
                    except Exception:
                        pass
                    newl.append(nop)
                si.on_wait = [waits[-1]]
            newl.append(inst)
        b.instructions = newl


def _build_nc():
    _patch_tail_drain()
    nc = bass.Bass("TRN2", target_bir_lowering=False, debug=False, num_devices=N_CORES)
    nop_cls = type(nc.sync.nop().ins)

    x_d = nc.dram_tensor("x", [TOK, E], BF16, kind="ExternalInput").ap()
    wqk_d = nc.dram_tensor("wqk", [E, 2 * E], BF16, kind="ExternalInput").ap()
    wv_d = nc.dram_tensor("wv", [E, E], BF16, kind="ExternalInput").ap()
    wp_d = nc.dram_tensor("wp", [E, E], BF16, kind="ExternalInput").ap()
    cos2_d = nc.dram_tensor("cos2", [P, LS], F32, kind="ExternalInput").ap()
    ss2_d = nc.dram_tensor("ss2", [P, LS], F32, kind="ExternalInput").ap()
    mask_d = nc.dram_tensor("mask", [P, P], F32, kind="ExternalInput").ap()
    w2v_d = nc.dram_tensor("w2v", [P, 8], BF16, kind="ExternalInput").ap()
    swap_d = nc.dram_tensor("swapmat", [P, P], BF16, kind="ExternalInput").ap()
    ident_d = nc.dram_tensor("ident", [P, P], BF16, kind="ExternalInput").ap()
    ident32_d = nc.dram_tensor("ident32", [P, P], F32, kind="ExternalInput").ap()
    y_d = nc.dram_tensor("y", [TOK, E], F32, kind="ExternalOutput").ap()

    KT = E // P  # 8 contraction tiles
    Exp = mybir.ActivationFunctionType.Exp
    Sqrt = mybir.ActivationFunctionType.Sqrt
    mult = mybir.AluOpType.mult

    with tile.TileContext(nc) as tc:
        with (
            tc.tile_pool(name="const", bufs=1) as cpool,
            tc.tile_pool(name="work", bufs=1) as wpool,
            tc.tile_pool(name="small", bufs=3) as spool,
            tc.tile_pool(name="single", bufs=1) as s1pool,
            tc.tile_pool(name="sq", bufs=3) as sqpool,
            tc.tile_pool(name="tmp", bufs=3) as tmppool,
            tc.tile_pool(name="pt", bufs=2) as ptpool,
            tc.tile_pool(name="mmps", bufs=2, space="PSUM") as mmps,
            tc.tile_pool(name="sumps", bufs=2, space="PSUM") as sumps,
            tc.tile_pool(name="sps", bufs=3, space="PSUM") as spsp,
            tc.tile_pool(name="ops", bufs=1, space="PSUM") as opsp,
        ):
            # ---- small constants first (cheap, unblock everything) ----
            ones64 = cpool.tile([1, 64], BF16, tag="ones64")
            nc.vector.memset(ones64[:], 1.0)
            ones128f = cpool.tile([1, P], F32, tag="ones128f")
            nc.vector.memset(ones128f[:], 1.0)
            cvals = cpool.tile([P, 3], F32, tag="cvals")
            nc.vector.memset(cvals[:, 0:1], 0.0)
            nc.vector.memset(cvals[:, 1:2], EPS)
            nc.vector.memset(cvals[:, 2:3], 64.0 * EPS)
            nc.const_aps.aps[(F32, 0.0)] = cvals[:, 0:1]
            nc.const_aps.aps[(F32, EPS)] = cvals[:, 1:2]
            nc.const_aps.aps[(F32, 64.0 * EPS)] = cvals[:, 2:3]

            # ---- resident loads, split per k-tile so compute starts early ----
            ident_sb = cpool.tile([P, P], BF16, tag="ident")
            nc.sync.dma_start(ident_sb[:], ident_d[:])
            xT = cpool.tile([P, KT, TOK], BF16, tag="xT")
            wqk_sb = cpool.tile([P, KT, 2 * E], BF16, tag="wqk")
            wv_sb = cpool.tile([P, KT, E], BF16, tag="wv")
            wp_sb = cpool.tile([P, KT, E], BF16, tag="wp")
            for tt in range(KT):
                xrow = sqpool.tile([P, E], BF16, tag="xrow")
                nc.sync.dma_start(xrow[:], x_d[tt * P : (tt + 1) * P, :])
                nc.scalar.dma_start(wv_sb[:, tt, :], wv_d[tt * P : (tt + 1) * P, :])
                for kk in range(KT):
                    tp = spsp.tile([P, P], BF16, tag="sps", name=f"tp{tt}_{kk}")
                    nc.tensor.transpose(tp[:], xrow[:, kk * P : (kk + 1) * P], ident_sb[:])
                    if kk % 2:
                        nc.scalar.copy(xT[:, kk, tt * P : (tt + 1) * P], tp[:])
                    else:
                        nc.vector.tensor_copy(xT[:, kk, tt * P : (tt + 1) * P], tp[:])
            w2v_sb = cpool.tile([P, 8], BF16, tag="w2v")
            nc.sync.dma_start(w2v_sb[:], w2v_d[:])
            for kk in range(KT):
                nc.scalar.dma_start(
                    wqk_sb[:, kk, :E], wqk_d[kk * P : (kk + 1) * P, :E]
                )
                nc.sync.dma_start(
                    wqk_sb[:, kk, E:], wqk_d[kk * P : (kk + 1) * P, E:]
                )
            ident32_sb = cpool.tile([P, P], F32, tag="ident32")
            nc.sync.dma_start(ident32_sb[:], ident32_d[:])
            swap_sb = cpool.tile([P, P], BF16, tag="swapmat")
            nc.sync.dma_start(swap_sb[:], swap_d[:])
            cos2_sb = cpool.tile([P, LS], F32, tag="cos2")
            nc.sync.dma_start(cos2_sb[:], cos2_d[:])
            ss2_sb = cpool.tile([P, LS], F32, tag="ss2")
            nc.sync.dma_start(ss2_sb[:], ss2_d[:])
            mask_sb = cpool.tile([P, P], F32, tag="mask")
            nc.sync.dma_start(mask_sb[:], mask_d[:])
            for kk in range(KT):
                nc.scalar.dma_start(wp_sb[:, kk, :], wp_d[kk * P : (kk + 1) * P, :])

            # ---- per-group state ----
            raw = {0: {}, 1: {}}
            rot = {0: {}, 1: {}}
            sums2 = {}
            sbq_sb = {}
            skT = {}
            v_sb = {0: {}, 1: {}}
            oT = {0: {}, 1: {}}

            def pass1_pair(g, mq):
                gs = g * LS
                if mq == 0:
                    sums2[g] = sumps.tile(
                        [33, LS], F32, tag="sums", name=f"sums2_{g}"
                    )
                sqs = {}
                for m in (mq, 8 + mq):
                    ps = mmps.tile([P, LS], F32, tag="mmps", name=f"qkps{g}_{m}")
                    for kk in range(KT):
                        nc.tensor.matmul(
                            ps[:],
                            wqk_sb[:, kk, m * P : (m + 1) * P],
                            xT[:, kk, gs : gs + LS],
                            start=(kk == 0),
                            stop=(kk == KT - 1),
                        )
                    sqs[m] = sqpool.tile([P, LS], BF16, tag="sq", name=f"sq{g}_{m}")
                    with nc.allow_low_precision(reason="bf16 squares for rms sums"):
                        nc.scalar.square(sqs[m][:], ps[:])
                    raw[g][m] = wpool.tile([P, LS], BF16, tag=f"raw{m}", name=f"raw{g}_{m}")
                    nc.scalar.copy(raw[g][m][:], ps[:])
                # adjacent col-disjoint sums matmuls (out rows 0 / 32) overlap on HW
                for m in (mq, 8 + mq):
                    srow = 0 if m < 8 else 32
                    nc.tensor.matmul(
                        sums2[g][srow : srow + 1, :],
                        w2v_sb[:, mq : mq + 1],
                        sqs[m][:],
                        start=(mq == 0),
                        stop=(mq == 7),
                        skip_group_check=True,
                    )

            def v_proj(g):
                gs = g * LS
                for mt in range(4):
                    v_sb[g][mt] = wpool.tile(
                        [P, NH * 65], BF16, tag=f"vsb{mt}", name=f"vsb{g}_{mt}"
                    )
                    nc.vector.memset(
                        v_sb[g][mt][:].rearrange("p (h c) -> p h c", c=65)[:, :, 64:65],
                        1.0,
                    )
                for mt in range(4):
                    for n in range(2):
                        ps = mmps.tile([P, LS], F32, tag="mmps", name=f"vps{g}_{mt}_{n}")
                        for kk in range(KT):
                            nc.tensor.matmul(
                                ps[:],
                                xT[:, kk, gs + mt * P : gs + (mt + 1) * P],
                                wv_sb[:, kk, n * LS : (n + 1) * LS],
                                start=(kk == 0),
                                stop=(kk == KT - 1),
                            )
                        dst = v_sb[g][mt][:, n * 520 : (n + 1) * 520].rearrange(
                            "p (h c) -> p h c", c=65
                        )[:, :, 0:64]
                        srcv = ps[:].rearrange("p (h c) -> p h c", c=64)
                        nc.vector.tensor_copy(dst, srcv)

            def scales_tables(g):
                # q: s_q = rsqrt(mean+eps) as a row, broadcast over partitions
                sv = s1pool.tile([1, LS], F32, tag="sv")
                nc.scalar.activation(
                    sv[:], sums2[g][0:1, :], Sqrt, bias=EPS, scale=1.0 / E
                )
                sq_row = spool.tile([1, LS], F32, tag="st", name=f"st{g}_q")
                nc.vector.reciprocal(sq_row[:], sv[:])
                sbq_ps = sumps.tile([P, LS], F32, tag="sums", name=f"sbqps{g}")
                nc.tensor.matmul(
                    sbq_ps[:], ones128f[:], sq_row[:], start=True, stop=True
                )
                sbq_sb[g] = s1pool.tile([P, LS], F32, tag="sbq", name=f"sbq{g}")
                nc.vector.tensor_copy(sbq_sb[g][:], sbq_ps[:])
                # k: 8*sqrt(mean+eps) as a row, transpose to columns, reciprocal
                svk = s1pool.tile([1, LS], F32, tag="svk")
                nc.scalar.activation(
                    svk[:], sums2[g][32:33, :], Sqrt, bias=64.0 * EPS, scale=64.0 / E
                )
                skps = sumps.tile([P, 4], F32, tag="sums", name=f"skps{g}")
                for i in range(4):
                    nc.tensor.transpose(
                        skps[:, i : i + 1],
                        svk[:, i * P : (i + 1) * P],
                        ident32_sb[0:1, 0:1],
                    )
                skT[g] = s1pool.tile([P, 4], F32, tag="skT", name=f"skT{g}")
                nc.vector.reciprocal(skT[g][:], skps[:])

            def qscale_pair(g, mq):
                nc.vector.tensor_tensor(
                    rot[g][mq][:], rot[g][mq][:], sbq_sb[g][:], mult
                )

            def rope_pair(g, mq):
                for m in (mq, 8 + mq):
                    cs, ss = cos2_sb[:], ss2_sb[:]
                    r = raw[g][m][:]
                    swp = sumps.tile([P, LS], F32, tag="sums", name=f"swp{g}_{m}")
                    nc.tensor.matmul(swp[:], swap_sb[:], r, start=True, stop=True)
                    tmp = tmppool.tile([P, LS], F32, tag="ropetmp")
                    nc.vector.tensor_mul(tmp[:], swp[:], ss)
                    tmp2 = tmppool.tile([P, LS], F32, tag="ropetmp2")
                    nc.gpsimd.tensor_mul(tmp2[:], r, cs)
                    rot[g][m] = wpool.tile([P, LS], BF16, tag=f"rot{m}", name=f"rot{g}_{m}")
                    nc.vector.tensor_add(rot[g][m][:], tmp[:], tmp2[:])

            def scores_pair(g, mq, state):
                ptA = ptpool.tile([P, 4, LS], BF16, tag="pt", name=f"pt{g}_{2*mq}")
                ptB = ptpool.tile([P, 4, LS], BF16, tag="pt", name=f"pt{g}_{2*mq+1}")
                state["pt0"], state["pt1"] = ptA, ptB
                for i in range(4):
                    qs = i * P
                    spsA = spsp.tile([P, LS], F32, tag="sps", name=f"spsA{g}_{mq}_{i}")
                    spsB = spsp.tile([P, LS], F32, tag="sps", name=f"spsB{g}_{mq}_{i}")
                    # adjacent K=64 matmuls on disjoint PE row groups (0-63 / 64-127):
                    # real hardware runs these concurrently (tile_position derives
                    # from base_partition); the cost model charges them serially.
                    nc.tensor.matmul(
                        spsA[:, qs:LS],
                        rot[g][8 + mq][0:64, qs : qs + P],
                        rot[g][mq][0:64, qs:LS],
                        start=True,
                        stop=True,
                    )
                    nc.tensor.matmul(
                        spsB[:, qs:LS],
                        rot[g][8 + mq][64:P, qs : qs + P],
                        rot[g][mq][64:P, qs:LS],
                        start=True,
                        stop=True,
                    )
                    for pt, sps in ((ptA, spsA), (ptB, spsB)):
                        nc.vector.tensor_add(
                            sps[:, qs : qs + P], sps[:, qs : qs + P], mask_sb[:]
                        )
                        nc.scalar.activation(
                            pt[:, i, qs:LS], sps[:, qs:LS], Exp,
                            scale=skT[g][:, i : i + 1],
                        )

            def attn_head(g, h, state):
                hp = (h % 2) * 64
                pt = state[f"pt{h % 2}"]
                ops = opsp.tile([65, LS], F32, tag="ops")
                for i in range(4):
                    qs = i * P
                    nc.tensor.matmul(
                        ops[:, qs:LS],
                        v_sb[g][i][:, h * 65 : (h + 1) * 65],
                        pt[:, i, qs:LS],
                        start=(i == 0),
                        stop=(i == 3),
                        skip_group_check=True,
                    )
                if hp == 0:
                    state["rts"] = spool.tile([1, 2, LS], BF16, tag="rts", name=f"rts{g}_{h}")
                    state["oraw"] = sqpool.tile([P, LS], F32, tag="oraw", name=f"oraw{g}_{h}")
                rts, oraw = state["rts"], state["oraw"]
                with nc.allow_low_precision(reason="bf16 softmax denominators"):
                    nc.vector.reciprocal(rts[:, h % 2, :], ops[64:65, :])
                nc.scalar.copy(oraw[hp : hp + 64, :], ops[0:64, :])

            def attn_pair_finish(g, mq, state):
                rts, oraw = state["rts"], state["oraw"]
                rb = sumps.tile([P, LS], F32, tag="sums", name=f"rb{g}_{mq}")
                nc.tensor.matmul(
                    rb[0:64, :], ones64[:], rts[:, 0, :], start=True, stop=True
                )
                nc.tensor.matmul(
                    rb[64:P, :], ones64[:], rts[:, 1, :], start=True, stop=True
                )
                oT[g][mq] = wpool.tile(
                    [P, LS], BF16, tag=f"oT{g}_{mq}", name=f"oT{g}_{mq}"
                )
                nc.vector.tensor_tensor(oT[g][mq][:], oraw[:], rb[:], mult)

            def proj_unit(g, j):
                gs = g * LS
                mt, n = j // 2, j % 2
                ps = mmps.tile([P, LS], F32, tag="mmps", name=f"yps{g}_{j}")
                for kk in range(KT):
                    nc.tensor.matmul(
                        ps[:],
                        oT[g][kk][:, mt * P : (mt + 1) * P],
                        wp_sb[:, kk, n * LS : (n + 1) * LS],
                        start=(kk == 0),
                        stop=(kk == KT - 1),
                    )
                ysb = sqpool.tile([P, LS], F32, tag="ysb")
                nc.scalar.copy(ysb[:], ps[:])
                nc.sync.dma_start(
                    y_d[gs + mt * P : gs + (mt + 1) * P, n * LS : (n + 1) * LS],
                    ysb[:],
                )

            # ---- software-pipelined schedule over the two groups ----
            v_proj(0)
            for mq in range(8):
                pass1_pair(0, mq)
            scales_tables(0)
            pend = []
            rope_pair(0, 0)
            qscale_pair(0, 0)
            for mq in range(8):
                st_h = {}
                scores_pair(0, mq, st_h)
                if mq < 7:
                    rope_pair(0, mq + 1)
                    qscale_pair(0, mq + 1)
                attn_head(0, 2 * mq, st_h)
                attn_head(0, 2 * mq + 1, st_h)
                pend.append((mq, st_h))
                if len(pend) > 2:
                    attn_pair_finish(0, *pend.pop(0))
                pass1_pair(1, mq)
            while pend:
                attn_pair_finish(0, *pend.pop(0))
            rope_pair(1, 0)
            rope_pair(1, 1)
            v_proj(1)
            scales_tables(1)
            for mq in range(8):
                qscale_pair(1, mq)
                st_h = {}
                scores_pair(1, mq, st_h)
                if 1 <= mq < 7:
                    rope_pair(1, mq + 1)
                attn_head(1, 2 * mq, st_h)
                attn_head(1, 2 * mq + 1, st_h)
                pend.append((mq, st_h))
                if len(pend) > 2:
                    attn_pair_finish(1, *pend.pop(0))
                proj_unit(0, mq)
            while pend:
                attn_pair_finish(1, *pend.pop(0))
            for j in range(8):
                proj_unit(1, j)
    _split_multi_waits(nc, nop_cls)
    return nc


def _get_nc():
    if "nc" not in _cache:
        _cache["nc"] = _build_nc()
    return _cache["nc"]


def _prep_inputs(x, W_qkv, ln_w, W_proj):
    bf = ml_dtypes.bfloat16
    W_qkv = np.asarray(W_qkv, np.float32)
    ln_w = np.asarray(ln_w, np.float32)
    if np.allclose(ln_w, 1.0):
        wqk = W_qkv[:, : 2 * E]
        w2inv = np.ones(E, np.float32)
    else:
        wqk = W_qkv[:, : 2 * E] * np.concatenate([ln_w, ln_w])[None, :]
        w2inv = 1.0 / (ln_w * ln_w)
    wqk = np.ascontiguousarray(wqk.astype(bf))
    wv = np.ascontiguousarray(W_qkv[:, 2 * E :].astype(bf))
    wp = np.ascontiguousarray(np.asarray(W_proj, np.float32).astype(bf))
    cos2, ss2 = _rope_tables()
    mask = np.where(
        np.arange(P)[:, None] > np.arange(P)[None, :], NEG_BIG, 0.0
    ).astype(np.float32)
    w2v = np.ascontiguousarray(w2inv.reshape(8, P).T.astype(bf))  # [P, 8]
    idx = np.arange(P)
    swapmat = (idx[:, None] == (idx[None, :] ^ 1)).astype(bf)  # adjacent-pair swap
    ident = np.eye(P).astype(bf)
    ident32 = np.eye(P, dtype=np.float32)

    xr = np.asarray(x, np.float32).reshape(B * NG, LS, E)
    shared = dict(
        wqk=wqk, wv=wv, wp=wp, cos2=cos2, ss2=ss2, mask=mask, w2v=w2v,
        swapmat=swapmat, ident=ident, ident32=ident32,
    )
    in_maps = []
    for c in range(N_CORES):
        xc = np.ascontiguousarray(xr[2 * c : 2 * c + 2].reshape(TOK, E).astype(bf))
        in_maps.append(dict(shared, x=xc))
    return in_maps


def kernel(x, W_qkv, ln_w, W_proj, _trace=False):
    nc = _get_nc()
    in_maps = _prep_inputs(x, W_qkv, ln_w, W_proj)
    res = bass_utils.run_bass_kernel_spmd(
        nc, in_maps, core_ids=list(range(N_CORES)), trace=_trace
    )
    ys = [r["y"] for r in res.results]  # each [TOK, E]
    out = np.stack(ys, axis=0).reshape(B, NG, LS, E)
    if _trace:
        kernel.last_results = res
    return out
